# revision 24
# baseline (speedup 1.0000x reference)
"""Trainium2 Bass kernel for a 2-layer bidirectional LSTM encoder.

Problem: x [256, 2048, 64] -> bilstm(H=4) -> [.,.,8] -> bilstm(H=2) -> [256, 2048, 4]

Strategy (8 cores, data parallel over batch, 32 seqs/core) — chunk-parallel
recurrence with ASYMMETRIC warmup: each direction only needs warmup on one
side (fwd left, bwd right), so chunks span just SPAN = 48 steps with K=16
warmup.  C=64 chunks at offsets with spacing pattern (32,32,31,32) — any 4
consecutive spacings sum to 127, so set-mates are uniformly strided in xT.
fwd owns local cols [K, K+s_c), bwd owns [0, s_c).  Layer 2 rides layer 1's
chunk grid; the h1 columns where the producing direction was still warming
(fwd cols [0,K), bwd cols [SPAN-K,SPAN)) are replaced by an SBUF DMA shuffle
importing the same times from the neighboring chunk (which owns them
accurately) before layer 2 starts.

Layout per stream: gates PSUM G [128, FS] per step, partitions = 4 quads
(i,f,o,g) x 32 rows (8 sets x H1; for layer 2: 8 sets x H2 x 2 directions
merged into one stream).  Free dim FS = 8 chunks-per-set x 32 batch = 256.
x is transposed/cast to fp16 on the host into xT [128 = 64 feats x 2
chunk-halves, TT*32] so a single strided-AP matmul per (pair, step) computes
all 8 chunk columns of a set pair.

Cell math (all fp16, PE accumulates fp32 in PSUM).  neuronxcc requires
equal base partitions when BOTH operands of a DVE op are in SBUF, so the
quad layout (i@0, f@32, o@64, g@96) is paired with state tiles at matching
bases (T1/U/D at base 32, TC at base 64); the g-quad's sigmoid ACT shifts
it from PSUM base 96 to SBUF base 0 (ACT may shift partitions freely):
  Gg  = sigmoid(2*G_g + 2b)  ACT, PSUM[96:128] -> SBUF@0; encodes
                             tanh(z) = 2*sigma(2z)-1
  SG  = sigmoid(G_ifo + b)   ACT, PSUM[0:96] -> SBUF@0
  T1  = (Gg - 0.5) * SG_i    = 0.5*i*g   (DVE stt, @0 -> out @32)
  U   = SG_f * D             (DVE tt 2x fp16, both @32)
  D   = U + T1               D tracks 0.5*c  (DVE tt 2x, @32)
  TC  = tanh(2*D)            ACT with scale=2, @32 -> @64
  H   = SG_o * TC            both @64 -> h1buf col @0
Engine ops are issued phase-grouped across the fwd/bwd streams so the
in-order engines never couple one stream's stall to the other.
"""

import numpy as np

_B, _S, _F = 256, 2048, 64
_H1, _H2 = 4, 2
_NC = 8
_BL = _B // _NC          # 32 seqs per core
_K = 16                  # warmup steps (one side per direction)
_C = 64                  # chunks per direction
_SPAN = 48               # steps per chunk
_SETS = 8
_CPS = _C // _SETS       # 8 chunks per set
_FS = _CPS * _BL         # 256 free cols per step
_W = 2                   # steps per PSUM window ([128, W*FS] fp32 <= 1 bank)
_NWIN = _SPAN // _W      # 24
_SPAT = (32, 32, 31, 32)  # spacing pattern; any 4 consecutive sum to 127
_SP = [_SPAT[c % 4] for c in range(_C - 1)]   # spacings between chunk starts
_A = [0]
for _s in _SP:
    _A.append(_A[-1] + _s)                    # chunk start offsets
_STRIDE = 127            # offset between set-mates (a[c+4]-a[c])
_HC = _C // 2            # 32 chunks per half
_UPOFF = _A[_HC]         # 1016: time offset of the upper half
_TT = _A[_HC - 1] + _SPAN  # 1032 time indices per half

assert _A[-1] + _SPAN == _S and _UPOFF + _TT == _S and _SPAN % _W == 0
assert all(_A[c + 4] - _A[c] == _STRIDE for c in range(_C - 4))

# quad order: i, f, o, g  (PyTorch blocks 0,1,3,2)
_QUADS = ((0, 0, 1.0), (1, 1, 1.0), (2, 3, 1.0), (3, 2, 1.0))


# ---------------------------------------------------------------- host-side
def _fp16(a):
    return np.asarray(a, np.float32).astype(np.float16)


def _pack_l1(Wih, Whh, bih, bhh):
    """lhsT tiles for one layer-1 direction."""
    Wih = np.asarray(Wih, np.float32)
    Whh = np.asarray(Whh, np.float32)
    bsum = (np.asarray(bih) + np.asarray(bhh)).astype(np.float32)
    wih_p = np.zeros((4, 128, 128), np.float32)   # per pair p: [K=128, M=128]
    whh = np.zeros((32, 128), np.float32)
    bias = np.zeros((128, 1), np.float32)
    for q, blk, sc in _QUADS:
        for s in range(_SETS):
            for k in range(_H1):
                col = 32 * q + 4 * s + k
                bias[col, 0] = sc * bsum[blk * _H1 + k]
                p, hi = s % 4, s >= 4
                wih_p[p, 64 * hi:64 * hi + 64, col] = Wih[blk * _H1 + k]
                whh[4 * s:4 * s + 4, col] = Whh[blk * _H1 + k]
    return wih_p, whh, bias


def _pack_l2(pf, pb):
    """lhsT tiles for layer 2, both directions merged into one stream.
    Row space of h2: 0:16 fwd (2s+k), 16:32 bwd.  Col space: 32q + 2s + k
    (fwd), 32q + 16 + 2s + k (bwd)."""
    w4 = np.zeros((4, 32, 128), np.float32)   # ff, fb, bf, bb: [K=32, M=128]
    whh = np.zeros((32, 128), np.float32)
    bias = np.zeros((128, 1), np.float32)
    for di, p in ((0, pf), (1, pb)):
        Wih = np.asarray(p["Wih"], np.float32)
        Whh = np.asarray(p["Whh"], np.float32)
        bsum = (np.asarray(p["bih"]) + np.asarray(p["bhh"])).astype(np.float32)
        for q, blk, sc in _QUADS:
            for s in range(_SETS):
                for k in range(_H2):
                    col = 32 * q + 16 * di + 2 * s + k
                    bias[col, 0] = sc * bsum[blk * _H2 + k]
                    w4[2 * di + 0, 4 * s:4 * s + 4, col] = Wih[blk * _H2 + k, 0:4]
                    w4[2 * di + 1, 4 * s:4 * s + 4, col] = Wih[blk * _H2 + k, 4:8]
                    whh[16 * di + 2 * s:16 * di + 2 * s + 2, col] = Whh[blk * _H2 + k]
    return w4, whh, bias


def _pack_weights(inp):
    out = {}
    for d, sfx in (("f", "_f"), ("b", "_b")):
        wih_p, whh, bi = _pack_l1(
            inp["l1_Wih" + sfx], inp["l1_Whh" + sfx],
            inp["l1_bih" + sfx], inp["l1_bhh" + sfx])
        for p in range(4):
            out[f"l1{d}_wih{p}"] = _fp16(wih_p[p])
        out[f"l1{d}_whh"] = _fp16(whh)
        out[f"l1{d}_bifo"] = bi[0:96]
        out[f"l1{d}_bg"] = bi[96:128]
    pf = {k: inp["l2_" + k + "_f"] for k in ("Wih", "Whh", "bih", "bhh")}
    pb = {k: inp["l2_" + k + "_b"] for k in ("Wih", "Whh", "bih", "bhh")}
    w4, whh2, bi2 = _pack_l2(pf, pb)
    for i, nm in enumerate(("ff", "fb", "bf", "bb")):
        out[f"l2_w{nm}"] = _fp16(w4[i])
    out["l2_whh"] = _fp16(whh2)
    out["l2_bifo"] = bi2[0:96]
    out["l2_bg"] = bi2[96:128]
    return out


def _wspec():
    import ml_dtypes  # noqa: F401
    f16 = np.float16
    spec = {}
    for d in ("f", "b"):
        for p in range(4):
            spec[f"l1{d}_wih{p}"] = ([128, 128], f16)
        spec[f"l1{d}_whh"] = ([32, 128], f16)
        spec[f"l1{d}_bifo"] = ([96, 1], np.float32)
        spec[f"l1{d}_bg"] = ([32, 1], np.float32)
    for nm in ("ff", "fb", "bf", "bb"):
        spec[f"l2_w{nm}"] = ([32, 128], f16)
    spec["l2_whh"] = ([32, 128], f16)
    spec["l2_bifo"] = ([96, 1], np.float32)
    spec["l2_bg"] = ([32, 1], np.float32)
    return spec


def _host_xt(xc):
    """xc: [BL, S, F] fp32 -> xT [128, TT*BL] fp16 (f + 64*half, tt, b)."""
    xt = np.zeros((128, _TT, _BL), np.float32)
    xT = np.transpose(xc, (2, 1, 0))                      # [F, S, B]
    xt[0:64, :, :] = xT[:, 0:_TT, :]
    xt[64:128, :, :] = xT[:, _UPOFF:_UPOFF + _TT, :]
    return _fp16(xt.reshape(128, _TT * _BL))


# ---------------------------------------------------------------- device
def _build(debug=False):
    import concourse.bacc as bacc
    import concourse.mybir as mybir
    from concourse.tile import TileContext
    from contextlib import ExitStack

    fp32 = mybir.dt.float32
    f16 = mybir.dt.float16
    Tanh = mybir.ActivationFunctionType.Tanh
    Sigm = mybir.ActivationFunctionType.Sigmoid
    Alu = mybir.AluOpType
    FS, W, SPAN = _FS, _W, _SPAN

    nc = bacc.Bacc(None, target_bir_lowering=False)
    nc._labels = {}
    nc._act_seq = []
    nc._dve_seq = []
    nc._mm_seq = []

    def _lb(inst, label):
        if label.startswith(("Gg", "SG", "TC")):
            nc._act_seq.append(label)
        elif label.startswith(("T1", "U_", "D_", "H_")):
            nc._dve_seq.append(label)
        elif label.startswith(("whh", "wih")):
            nc._mm_seq.append(label)
        return inst
    xTd = nc.dram_tensor("xT", [128, _TT * _BL], f16, kind="ExternalInput")
    outd = nc.dram_tensor("out2", [32, SPAN * FS], f16, kind="ExternalOutput")
    np_to_bir = {np.dtype(np.float32): fp32, np.dtype(np.float16): f16}
    wdram = {}
    for k, (shp, dt) in _wspec().items():
        wdram[k] = nc.dram_tensor(k, shp, np_to_bir[np.dtype(dt)],
                                  kind="ExternalInput")
    if debug:
        h1d = {d: nc.dram_tensor(f"h1{d}_dbg", [32, SPAN * FS], f16,
                                 kind="ExternalOutput") for d in ("f", "b")}

    with TileContext(nc) as tc, ExitStack() as ctx:
        xpool = ctx.enter_context(tc.tile_pool(name="xpool", bufs=1))
        wpool = ctx.enter_context(tc.tile_pool(name="wpool", bufs=1))
        hpool = ctx.enter_context(tc.tile_pool(name="hpool", bufs=1))
        spool = ctx.enter_context(tc.tile_pool(name="spool", bufs=1))
        opool = ctx.enter_context(tc.tile_pool(name="opool", bufs=4))
        ppool = ctx.enter_context(tc.tile_pool(name="ppool", bufs=6,
                                               space="PSUM"))

        # xT first: it is the long-pole DMA and gates the first matmuls
        xT = xpool.tile([128, _TT * _BL], f16, name="xT")
        nc.sync.dma_start(xT[:], xTd[:])
        xTv = xT[:].rearrange("p (t b) -> p t b", b=_BL)

        wtile = {}
        for k, (shp, dt) in _wspec().items():
            t = wpool.tile(shp, np_to_bir[np.dtype(dt)], name=k)
            nc.sync.dma_start(t[:], wdram[k][:])
            wtile[k] = t

        # resident h~1 buffers (fp16): [32 = 8 sets x H1, SPAN*FS]
        h1buf = {d: hpool.tile([32, SPAN * FS], f16, name=f"h1{d}")
                 for d in ("f", "b")}

        def make_state(tag):
            # Gg/SG at base 0; T1/U/D live at base 32 and TC at base 64 of
            # wider tiles so DVE operand pairs share base partitions.
            # ACT-written tiles (SG/Gg/TC) are ping-ponged by step parity so
            # their write-after-read sems resolve a step early — otherwise
            # each gets a 2nd wait and a SEQ-blocking EventSemaphore.
            st = {}
            for pp in (0, 1):
                st[f"SG{pp}"] = spool.tile([96, FS], f16, name=f"SG{pp}{tag}")
                st[f"Gg{pp}"] = spool.tile([32, FS], f16, name=f"Gg{pp}{tag}")
                st[f"TC{pp}"] = spool.tile([96, FS], f16, name=f"TC{pp}{tag}")
            for k in ("T1", "U", "D"):
                st[k] = spool.tile([64, FS], f16, name=f"{k}{tag}")
            return st

        # ---------------- layer 1 ----------------
        def l1_wih_window(d, n, G):
            """Fill PSUM window n of stream d with input projections.
            One strided-AP matmul per (pair, step) covers all 7 chunks."""
            for jw in range(W):
                j = n * W + jw
                for p in range(4):
                    base = _A[p] + (j if d == "f" else SPAN - 1 - j)
                    rhs = xTv[:, base:base + _STRIDE * (_CPS - 1) + 1:_STRIDE, :]
                    _lb(nc.tensor.matmul(G[:, jw * FS:(jw + 1) * FS],
                                         wtile[f"l1{d}_wih{p}"][:, :], rhs,
                                         start=(jw == 0 and p == 0),
                                         stop=False), f"wih_{d}{n}.{jw}")

        st1 = {d: make_state("1" + d) for d in ("f", "b")}
        gwin = {d: [None] * (_NWIN + 1) for d in ("f", "b")}
        for d in ("f", "b"):
            gwin[d][0] = ppool.tile([128, W * FS], fp32, name=f"g{d}", tag="gw")
            l1_wih_window(d, 0, gwin[d][0])

        def h1col(d, j):
            c = j if d == "f" else SPAN - 1 - j
            return h1buf[d][:, c * FS:(c + 1) * FS]

        for n in range(_NWIN):
            for d in ("f", "b"):
                if n + 1 < _NWIN:
                    gwin[d][n + 1] = ppool.tile([128, W * FS], fp32,
                                                name=f"g{d}", tag="gw")
                    l1_wih_window(d, n + 1, gwin[d][n + 1])
            for jw in range(W):
                j = n * W + jw
                sl = {d: gwin[d][n][:, jw * FS:(jw + 1) * FS] for d in ("f", "b")}
                if j > 0:
                    for d in ("f", "b"):
                        _lb(nc.tensor.matmul(sl[d], wtile[f"l1{d}_whh"][:, :],
                                             h1col(d, j - 1),
                                             start=False, stop=(jw == W - 1)),
                            f"whh_{d}{j}")
                pp = j % 2
                for d in ("f", "b"):
                    s = st1[d]
                    _lb(nc.scalar.activation(s[f"Gg{pp}"][:, :],
                                             sl[d][96:128, :], Tanh,
                                             bias=wtile[f"l1{d}_bg"][:, :]),
                        f"Gg_{d}{j}")
                    _lb(nc.scalar.activation(s[f"SG{pp}"][:, :], sl[d][0:96, :],
                                             Sigm,
                                             bias=wtile[f"l1{d}_bifo"][:, :]),
                        f"SG_{d}{j}")
                for d in ("f", "b"):
                    s = st1[d]
                    if j == 0:
                        _lb(nc.vector.tensor_tensor(
                            s["D"][32:64, :], s[f"Gg{pp}"][:, :],
                            s[f"SG{pp}"][0:32, :], Alu.mult), f"D_{d}{j}")
                    else:
                        _lb(nc.vector.tensor_tensor(
                            s["T1"][32:64, :], s[f"Gg{pp}"][:, :],
                            s[f"SG{pp}"][0:32, :], Alu.mult), f"T1_{d}{j}")
                        _lb(nc.vector.tensor_tensor(
                            s["U"][32:64, :], s[f"SG{pp}"][32:64, :],
                            s["D"][32:64, :], Alu.mult), f"U_{d}{j}")
                        _lb(nc.vector.tensor_tensor(
                            s["D"][32:64, :], s["U"][32:64, :],
                            s["T1"][32:64, :], Alu.add), f"D_{d}{j}")
                for d in ("f", "b"):
                    s = st1[d]
                    _lb(nc.scalar.activation(s[f"TC{pp}"][64:96, :],
                                             s["D"][32:64, :], Tanh),
                        f"TC_{d}{j}")
                for d in ("f", "b"):
                    s = st1[d]
                    _lb(nc.vector.tensor_tensor(h1col(d, j),
                                                s[f"SG{pp}"][64:96, :],
                                                s[f"TC{pp}"][64:96, :],
                                                Alu.mult), f"H_{d}{j}")

        if debug:
            for d in ("f", "b"):
                nc.sync.dma_start(h1d[d][:], h1buf[d][:])

        # ------- shuffle: import neighbor-chunk h1 for l2's edge columns --
        # h1fx[(s,cs), u<K]  = h1f of chunk c-1 at col u+sp[c-1] (chunk 0:
        # its own col u, which is exact).  h1bx[(s,cs), u] (u = col-(SPAN-K))
        # = h1b of chunk c+1 at col u+(32-sp[c]) (chunk C-1: own col).
        h1fx = hpool.tile([32, _K * FS], f16, name="h1fx")
        h1bx = hpool.tile([32, _K * FS], f16, name="h1bx")
        fxv = h1fx[:].rearrange("p (t f) -> p t f", f=FS)
        bxv = h1bx[:].rearrange("p (t f) -> p t f", f=FS)
        h1v = {d: h1buf[d][:].rearrange("p (t f) -> p t f", f=FS)
               for d in ("f", "b")}

        def shuf2(dst, dsl, ssrc, ssl, t0):
            nc.sync.dma_start(dst[dsl[0]:dsl[1], 0:_K, dsl[2]:dsl[3]],
                              h1v[ssrc][ssl[0]:ssl[1], t0:t0 + _K,
                                        ssl[2]:ssl[3]])

        # h1fx (dest set s <- src chunk c-1): bulk row-block moves
        shuf2(fxv, (4, 12, 0, FS), "f", (0, 8, 0, FS), 32)     # s=1,2
        shuf2(fxv, (12, 16, 0, FS), "f", (8, 12, 0, FS), 31)   # s=3
        shuf2(fxv, (20, 28, 0, FS), "f", (16, 24, 0, FS), 32)  # s=5,6
        shuf2(fxv, (28, 32, 0, FS), "f", (24, 28, 0, FS), 31)  # s=7
        shuf2(fxv, (0, 4, _BL, FS), "f", (12, 16, 0, FS - _BL), 32)   # s=0,cs>0
        shuf2(fxv, (16, 20, _BL, FS), "f", (28, 32, 0, FS - _BL), 32)  # s=4,cs>0
        shuf2(fxv, (0, 4, 0, _BL), "f", (0, 4, 0, _BL), 0)     # chunk 0 self
        shuf2(fxv, (16, 20, 0, _BL), "f", (12, 16, (_CPS - 1) * _BL, FS), 32)  # c=32<-31
        # h1bx (dest set s <- src chunk c+1); src col base 32-sp[c]+ (SPAN-K- s?)
        # natural dest cols are [SPAN-K, SPAN); src col = u' + (32 - sp[c]).
        shuf2(bxv, (0, 8, 0, FS), "b", (4, 12, 0, FS), 0)      # s=0,1 (sp=32)
        shuf2(bxv, (8, 12, 0, FS), "b", (12, 16, 0, FS), 1)    # s=2 (sp=31)
        shuf2(bxv, (12, 16, 0, (_CPS - 1) * _BL), "b", (0, 4, _BL, FS), 0)  # s=3,cs<7
        shuf2(bxv, (16, 24, 0, FS), "b", (20, 28, 0, FS), 0)   # s=4,5
        shuf2(bxv, (24, 28, 0, FS), "b", (28, 32, 0, FS), 1)   # s=6 (sp=31)
        shuf2(bxv, (28, 32, 0, (_CPS - 1) * _BL), "b", (16, 20, _BL, FS), 0)  # s=7,cs<7
        shuf2(bxv, (12, 16, (_CPS - 1) * _BL, FS), "b", (16, 20, 0, _BL), 0)  # c=31<-32
        shuf2(bxv, (28, 32, (_CPS - 1) * _BL, FS), "b",
              (28, 32, (_CPS - 1) * _BL, FS), SPAN - _K)       # chunk 63 self

        # ---------------- layer 2 (both directions in one stream) --------
        st2 = make_state("2")
        hprev = None
        g2win = [None] * (_NWIN + 1)

        def l2src(d, col):
            if d == "f" and col < _K:
                return h1fx[:, col * FS:(col + 1) * FS]
            if d == "b" and col >= SPAN - _K:
                u = col - (SPAN - _K)
                return h1bx[:, u * FS:(u + 1) * FS]
            return h1buf[d][:, col * FS:(col + 1) * FS]

        def l2_wih_window(n, G):
            for jw in range(W):
                j = n * W + jw
                cf, cb = j, SPAN - 1 - j
                for i, (nm, src, col) in enumerate((
                        ("ff", "f", cf), ("fb", "b", cf),
                        ("bf", "f", cb), ("bb", "b", cb))):
                    _lb(nc.tensor.matmul(
                        G[:, jw * FS:(jw + 1) * FS],
                        wtile[f"l2_w{nm}"][:, :], l2src(src, col),
                        start=(jw == 0 and i == 0), stop=False),
                        f"wih2_{nm}{n}.{jw}")

        g2win[0] = ppool.tile([128, W * FS], fp32, name="g2", tag="gw")
        l2_wih_window(0, g2win[0])
        for n in range(_NWIN):
            if n + 1 < _NWIN:
                g2win[n + 1] = ppool.tile([128, W * FS], fp32,
                                          name="g2", tag="gw")
                l2_wih_window(n + 1, g2win[n + 1])
            ost = opool.tile([32, W * FS], f16, name="ost", tag="ost")
            for jw in range(W):
                j = n * W + jw
                sl = g2win[n][:, jw * FS:(jw + 1) * FS]
                if j > 0:
                    hp = hprev if jw == 0 else ost[:, (jw - 1) * FS:jw * FS]
                    _lb(nc.tensor.matmul(sl, wtile["l2_whh"][:, :], hp,
                                         start=False, stop=(jw == W - 1)),
                        f"whh2_{j}")
                s = st2
                pp = j % 2
                _lb(nc.scalar.activation(s[f"Gg{pp}"][:, :], sl[96:128, :],
                                         Tanh, bias=wtile["l2_bg"][:, :]),
                    f"Gg_2{j}")
                _lb(nc.scalar.activation(s[f"SG{pp}"][:, :], sl[0:96, :], Sigm,
                                         bias=wtile["l2_bifo"][:, :]),
                    f"SG_2{j}")
                if j == 0:
                    _lb(nc.vector.tensor_tensor(
                        s["D"][32:64, :], s[f"Gg{pp}"][:, :],
                        s[f"SG{pp}"][0:32, :], Alu.mult), f"D_2{j}")
                else:
                    _lb(nc.vector.tensor_tensor(
                        s["T1"][32:64, :], s[f"Gg{pp}"][:, :],
                        s[f"SG{pp}"][0:32, :], Alu.mult), f"T1_2{j}")
                    _lb(nc.vector.tensor_tensor(
                        s["U"][32:64, :], s[f"SG{pp}"][32:64, :],
                        s["D"][32:64, :], Alu.mult), f"U_2{j}")
                    _lb(nc.vector.tensor_tensor(
                        s["D"][32:64, :], s["U"][32:64, :], s["T1"][32:64, :],
                        Alu.add), f"D_2{j}")
                _lb(nc.scalar.activation(s[f"TC{pp}"][64:96, :],
                                         s["D"][32:64, :], Tanh), f"TC_2{j}")
                _lb(nc.vector.tensor_tensor(ost[:, jw * FS:(jw + 1) * FS],
                                            s[f"SG{pp}"][64:96, :],
                                            s[f"TC{pp}"][64:96, :], Alu.mult),
                    f"H_2{j}")
            hprev = ost[:, (W - 1) * FS:W * FS]
            nc.sync.dma_start(outd[:, n * W * FS:(n + 1) * W * FS], ost[:])
    nc.finalize()
    return nc


# ---------------------------------------------------------------- entry
def _chunk_set_cs(c):
    if c < _HC:
        return c % 4, c // 4
    return 4 + (c - _HC) % 4, (c - _HC) // 4


def _assemble(o2):
    """o2: [32, SPAN, FS] fp32 per core -> [BL, S, 4] canonical fp32.
    fwd owns local cols [K, K+s_c) (chunk 0: [0, ...)); bwd owns [0, s_c)
    (chunk C-1: [0, SPAN)).  bwd values for local time u sit at step col
    SPAN-1-u."""
    out = np.zeros((_BL, _S, 4), np.float32)
    for c in range(_C):
        s, cs = _chunk_set_cs(c)
        sc = _SP[c] if c < _C - 1 else _SPAN - _K
        fj0 = 0 if c == 0 else _K
        fj1 = _K + sc
        blk_f = o2[2 * s:2 * s + 2, fj0:fj1, cs * _BL:(cs + 1) * _BL]
        out[:, _A[c] + fj0:_A[c] + fj1, 0:2] = np.transpose(blk_f, (2, 1, 0))
        bj1 = _SPAN if c == _C - 1 else _SP[c]
        blk_b = o2[16 + 2 * s:16 + 2 * s + 2, ::-1, cs * _BL:(cs + 1) * _BL]
        blk_b = blk_b[:, 0:bj1]
        out[:, _A[c]:_A[c] + bj1, 2:4] = np.transpose(blk_b, (2, 1, 0))
    return out


def _run(x_full, packed, n_cores, _return_res=False, **runkw):
    from concourse.bass_utils import run_bass_kernel_spmd
    nc = _build()
    in_maps = []
    for c in range(n_cores):
        m = dict(packed)
        m["xT"] = _host_xt(np.asarray(x_full[c * _BL:(c + 1) * _BL],
                                      np.float32))
        in_maps.append(m)
    res = run_bass_kernel_spmd(nc, in_maps, core_ids=list(range(n_cores)),
                               **runkw)
    out = np.zeros((n_cores * _BL, _S, 4), np.float32)
    for c in range(n_cores):
        r = res.results[c]
        o2 = np.asarray(r["out2"], np.float32).reshape(32, _SPAN, _FS)
        out[c * _BL:(c + 1) * _BL] = _assemble(o2)
    if _return_res:
        return out, res
    return out


def kernel(**inputs):
    packed = _pack_weights(inputs)
    x = np.asarray(inputs["x"], np.float32)
    return _run(x, packed, _NC)
